# revision 18
# baseline (speedup 1.0000x reference)
"""Trainium2 Bass kernel for nn_CrossAttentionBlock.

Reference computation (per batch b):
  Q = wq @ x1   [32, 4096]     (x1 = feat1[b] reshaped [256, HW])
  K = wk @ x2   [32, 4096]
  V = wv @ x2   [256, 4096]
  A = softmax_j(Q^T K / sqrt(32))      [4096, 4096]
  out[c, i] = sum_j V[c, j] A[i, j]    [256, 4096]

Sharding: 8 cores = 4 batches x 2 query-halves (2048 queries each).
Each core gets x1 = feat1[b][:, half] and the full x2 = feat2[b].

Layout: keys (j) in the partition dimension everywhere, no transposes:
  S^T[j, i] tiles via matmul(lhsT=K[:, jtile], rhs=Q[:, ichunk]), bf16
      operands zero-padded to 128 partitions (K=128 matmuls are ~2x
      faster than K=32 on TRN2; all inputs cast to bf16 on the host so
      every weight load is 2-byte/fast).
  E8 = exp(S^T*scale - SHIFT) emitted DIRECTLY as fp8e4m3 by the ACT
      engine (the constant logit shift cancels in the softmax ratio and
      keeps E8 in e4m3's normal range).
  numerator += DoubleRow-matmul(lhsT=V'8 pair tile, rhs=E8 pair)
      where V'8 = fp8([V ; ones]) stored as [128, 2, 257] jtile-pair
      tiles. fp8 DoubleRow contracts 2x128 keys per instruction at the
      same stream rate as bf16 -> 2x AV throughput (measured 216ns for
      a 512-wide DR matmul, same as bf16). The c-slices are 80/80/97
      rows; the denominator is row 96 of the third slice (ones column).
  epilogue per chunk: av psum -> SBUF copies first (frees the psum
      banks for the next chunk), then 1/denom + broadcast + multiply
      run off the critical path, emitted during the next chunk.
x1/x2 are DMA'd in 512-column pieces so the next loop iteration's
loads stream in behind the epilogue instead of stalling the PE.
Measured rel err ~1.5e-2 vs the fp32 reference (fp8 quantization of E
and V dominates; harness gate is 2e-2).
"""

import numpy as np

import concourse.bass as bass
import concourse.tile as tile
from concourse import bacc, mybir
from concourse._compat import axon_active

f32 = mybir.dt.float32
f32r = mybir.dt.float32r
bf16 = mybir.dt.bfloat16
fp8 = mybir.dt.float8e4

B, C, H, W = 4, 256, 64, 64
HW = H * W            # 4096
D = 32                # q/k channels
NCORES = 8
IHALF = HW // 2       # 2048 queries per core
NI = 512              # query chunk (psum bank)
NJT = HW // 128       # 32 key tiles
NIC = IHALF // NI     # 4 query chunks
GRP = 2               # jtiles per exp batch (one fp8 DoubleRow pair)
NPAIR = NJT // GRP    # 16 jtile pairs
SCALE = 1.0 / np.sqrt(np.float32(D))
SHIFT = 1.5           # logit shift: keeps E8 in e4m3's normal range

_CACHE = {}
Exp = mybir.ActivationFunctionType.Exp
AluAdd = mybir.AluOpType.add
DR = mybir.MatmulPerfMode.DoubleRow

SL = [96, 96, 80]       # AV slice rows (DR weights need multiple-of-16 cols)
SLO = [0, 96, 192]      # channel offset of each slice; last = 64 ch + 16 ones
DEN = C - SLO[2]        # denominator row within slice 3 (= 64)


def _build(has_bv: bool, loop_n: int = 1):
    nc = bacc.Bacc("TRN2", target_bir_lowering=False, debug=False, num_devices=NCORES)

    x1 = nc.dram_tensor("x1", [C, IHALF], bf16, kind="ExternalInput").ap()
    x2 = nc.dram_tensor("x2", [C, HW], bf16, kind="ExternalInput").ap()
    wqT = nc.dram_tensor("wqT", [C, 128], bf16, kind="ExternalInput").ap()
    wkT = nc.dram_tensor("wkT", [C, 128], bf16, kind="ExternalInput").ap()
    wvT = nc.dram_tensor("wvT", [C, C], bf16, kind="ExternalInput").ap()
    bq = nc.dram_tensor("bq", [128, 1], f32, kind="ExternalInput").ap()
    bk = nc.dram_tensor("bk", [128, 1], f32, kind="ExternalInput").ap()
    bv = nc.dram_tensor("bv", [1, C], bf16, kind="ExternalInput").ap()
    out = nc.dram_tensor("out", [C, IHALF], f32, kind="ExternalOutput").ap()

    with tile.TileContext(nc) as tc:
        with tc.tile_pool(name="persist", bufs=1) as per, \
             tc.tile_pool(name="xpool", bufs=1) as xp, \
             tc.tile_pool(name="qk", bufs=1) as qkp, \
             tc.tile_pool(name="ps", bufs=1, space="PSUM") as ps, \
             tc.tile_pool(name="epool", bufs=3) as epool, \
             tc.tile_pool(name="opool", bufs=3) as opool:
            # constants (loaded once, outside any timing loop)
            bq_sb = per.tile([128, 1], f32, tag="bq", name="bq")
            bk_sb = per.tile([128, 1], f32, tag="bk", name="bk")
            bv_sb = per.tile([1, C], bf16, tag="bv", name="bv")
            nc.sync.dma_start(bq_sb[:], bq[:])
            nc.sync.dma_start(bk_sb[:], bk[:])
            if has_bv:
                nc.sync.dma_start(bv_sb[:], bv[:])
            nshift = per.tile([128, 1], f32, tag="nshift", name="nshift")
            nc.vector.memset(nshift[:], -float(SHIFT))
            ones_pf = per.tile([128, 16], f32, tag="ones_pf", name="ones_pf")
            nc.vector.memset(ones_pf[:], 1.0)
            ones_rowf = per.tile([1, 128], f32, tag="ones_rowf", name="ones_rowf")
            nc.vector.memset(ones_rowf[:], 1.0)
            ones_row = per.tile([1, 128], f32r, tag="ones_row", name="ones_row")
            nc.vector.tensor_copy(ones_row[:], ones_rowf[:])
            ones_brow = per.tile([1, 128], bf16, tag="ones_brow", name="ones_brow")
            nc.vector.tensor_copy(ones_brow[:], ones_rowf[:])
            vt_sb = [[per.tile([128, 2, SL[ct]], fp8, tag=f"vt{g}_{ct}",
                                name=f"vt{g}_{ct}") for ct in range(3)]
                     for g in range(NPAIR)]
            for g in range(NPAIR):
                vf = vt_sb[g][2][:].rearrange("p a b -> p (a b)")
                rows = C - SLO[2]
                for p in range(2):
                    nc.vector.tensor_copy(
                        vf[:, p * SL[2] + rows:(p + 1) * SL[2]],
                        ones_pf[:, 0:SL[2] - rows])
            wq_sb = [per.tile([128, 128], bf16, tag=f"wq_{k}", name=f"wq_{k}") for k in range(2)]
            wk_sb = [per.tile([128, 128], bf16, tag=f"wk_{k}", name=f"wk_{k}") for k in range(2)]
            wv_sb = [per.tile([128, C], bf16, tag=f"wv_{k}", name=f"wv_{k}") for k in range(2)]
            for k in range(2):
                sl = slice(128 * k, 128 * (k + 1))
                nc.sync.dma_start(wq_sb[k][:], wqT[sl, :])
                nc.sync.dma_start(wk_sb[k][:], wkT[sl, :])
                nc.sync.dma_start(wv_sb[k][:], wvT[sl, :])

            def load(x1_sb, x2_sb):
                # activations in 2048-col pieces spread over the 3 DMA queues
                NX = 2048
                queues = [nc.sync, nc.gpsimd, nc.sync, nc.gpsimd]
                qi = 0
                for c8 in range(HW // NX):
                    cs = slice(NX * c8, NX * (c8 + 1))
                    for k in range(2):
                        sl = slice(128 * k, 128 * (k + 1))
                        queues[qi % 4].dma_start(x2_sb[k][:, cs], x2[sl, cs])
                        qi += 1
                for k in range(2):
                    sl = slice(128 * k, 128 * (k + 1))
                    nc.scalar.dma_start(x1_sb[k][:], x1[sl, :])

            def compute(x1_sb, x2_sb, carried_finish=None):
                # Q/K zero-padded to 128 partitions (rows 32..127 = 0)
                q_sb = qkp.tile([128, IHALF], bf16, tag="q", name="q")
                k_sb = qkp.tile([128, HW], bf16, tag="k", name="k")

                # V^T into persistent per-slice pair tiles [128 j, 2, SL] fp8
                # (contiguous pair blocks: DR ldweights needs full-tile APs;
                # ones rows of the last slice are pre-written at setup).
                # psum slots borrowed from the av banks (free during proj);
                # casts split DVE/ACT so neither engine paces the PE.
                Copy = mybir.ActivationFunctionType.Copy
                for t in range(NJT):
                    pv = ps.tile([128, C], f32, tag=f"av{t % 3}", bufs=1,
                                 name="pv", padded_shape=[128, NI])
                    js = slice(128 * t, 128 * (t + 1))
                    nc.tensor.matmul(pv[:], x2_sb[0][:, js], wv_sb[0][:],
                                     start=True, stop=False)
                    nc.tensor.matmul(pv[:], x2_sb[1][:, js], wv_sb[1][:],
                                     start=False, stop=not has_bv)
                    if has_bv:
                        nc.tensor.matmul(pv[:], ones_brow[:], bv_sb[:],
                                         start=False, stop=True)
                    p = t % 2
                    vts = vt_sb[t // 2]
                    for ct in range(3):
                        vf = vts[ct][:].rearrange("p a b -> p (a b)")
                        a, rows = SLO[ct], min(SL[ct], C - SLO[ct])
                        dst = vf[:, p * SL[ct]:p * SL[ct] + rows]
                        if ct == 2:
                            nc.scalar.activation(dst, pv[:, a:a + rows], Copy)
                        else:
                            nc.vector.tensor_copy(dst, pv[:, a:a + rows])

                # K then Q: [32, *] in chunks of 512, bias-added on DVE
                for ic in range(HW // NI):
                    pk = ps.tile([128, NI], f32, tag=f"av{ic % 3}", bufs=1, name="pk")
                    cs = slice(NI * ic, NI * (ic + 1))
                    nc.tensor.matmul(pk[:], wk_sb[0][:], x2_sb[0][:, cs],
                                     start=True, stop=False)
                    nc.tensor.matmul(pk[:], wk_sb[1][:], x2_sb[1][:, cs],
                                     start=False, stop=True)
                    nc.vector.tensor_scalar(k_sb[:, cs], pk[:], bk_sb[:, 0:1], None, AluAdd)
                for ic in range(NIC):
                    pq = ps.tile([128, NI], f32, tag=f"av{ic % 3}", bufs=1, name="pq")
                    cs = slice(NI * ic, NI * (ic + 1))
                    nc.tensor.matmul(pq[:], wq_sb[0][:], x1_sb[0][:, cs],
                                     start=True, stop=False)
                    nc.tensor.matmul(pq[:], wq_sb[1][:], x1_sb[1][:, cs],
                                     start=False, stop=True)
                    nc.vector.tensor_scalar(q_sb[:, cs], pq[:], bq_sb[:, 0:1], None, AluAdd)

                # --- attention ---

                def epilogue(ic, av):
                    # free the av psum banks ASAP: copy raw numerators to SBUF
                    o_raw = [None, None, None]
                    for ct in (0, 1, 2):
                        rows = min(SL[ct], C - SLO[ct]) + (1 if ct == 2 else 0)
                        orw = opool.tile([rows, NI], f32, tag=f"or{ct}", name="orw")
                        nc.vector.tensor_copy(orw[:], av[ct][0:rows, :])
                        o_raw[ct] = orw
                    # 1/denom -> broadcast -> multiply: entirely off the PE
                    recip = opool.tile([1, NI], f32, tag="recip", name="recip")
                    nc.vector.reciprocal(recip[:], o_raw[2][DEN:DEN + 1, :])

                    def finish():
                        bc_sb = opool.tile([128, NI], f32, tag="bc_sb", name="bc_sb")
                        nc.gpsimd.partition_broadcast(bc_sb[:], recip[:])
                        for ct in range(3):
                            a = SLO[ct]
                            rows = min(SL[ct], C - a)
                            o = opool.tile([rows, NI], f32, tag=f"o{ct}", name="o")
                            nc.vector.tensor_mul(o[:], o_raw[ct][0:rows, :],
                                                 bc_sb[0:rows, :])
                            nc.gpsimd.dma_start(
                                out[a:a + rows, NI * ic:NI * (ic + 1)], o[:])
                    return finish

                if carried_finish is not None:
                    carried_finish()   # prev compute's last-chunk output

                finish_last = None
                for ic in range(NIC):
                    qs = q_sb[:, NI * ic:NI * (ic + 1)]
                    av = [ps.tile([SL[ct], NI], f32, tag=f"av{ct}", bufs=1,
                                  name=f"av{ct}")
                          for ct in range(3)]

                    def emit_av(e, g):
                        ep = e[:].rearrange("p (a b) -> p a b", a=2)
                        first, last = g == 0, g == NPAIR - 1
                        for ct in range(3):
                            nc.tensor.matmul(av[ct][:], vt_sb[g][ct][:], ep,
                                             start=first, stop=last, perf_mode=DR)

                    prevs = []
                    for g in range(NPAIR):
                        st = ps.tile([128, GRP * NI], f32, tag="st", bufs=2, name="st")
                        for k in range(GRP):
                            t = g * GRP + k
                            nc.tensor.matmul(st[:, NI * k:NI * (k + 1)],
                                             k_sb[:, 128 * t:128 * (t + 1)], qs,
                                             start=True, stop=True)
                        if len(prevs) == 2:
                            emit_av(*prevs.pop(0))
                        e = epool.tile([128, GRP * NI], fp8, tag="e", name="e")
                        nc.scalar.activation(e[:], st[:], Exp, scale=float(SCALE),
                                             bias=nshift[:])
                        prevs.append((e, g))
                    for p in prevs:
                        emit_av(*p)

                    fin = epilogue(ic, av)
                    if ic == NIC - 1:
                        finish_last = fin
                    else:
                        fin()
                return finish_last

            x_a = [[xp.tile([128, IHALF], bf16, tag=f"xa1_{k}", name=f"xa1_{k}")
                    for k in range(2)],
                   [xp.tile([128, HW], bf16, tag=f"xa2_{k}", name=f"xa2_{k}")
                    for k in range(2)]]
            if loop_n == 1:
                load(*x_a)
                fin = compute(*x_a)
                fin()
            else:
                # 2 logical iterations per hardware-loop pass, software-
                # pipelined: x_b loads while compute(x_a) runs and vice
                # versa, so input DMA never stalls the PE in steady state.
                x_b = [[xp.tile([128, IHALF], bf16, tag=f"xb1_{k}", name=f"xb1_{k}")
                        for k in range(2)],
                       [xp.tile([128, HW], bf16, tag=f"xb2_{k}", name=f"xb2_{k}")
                        for k in range(2)]]
                load(*x_a)     # prologue for the first pass
                with tc.For_i(0, loop_n, 1, hint_engines=(mybir.EngineType.PE,
                                                          mybir.EngineType.Activation)):
                    load(*x_b)
                    fin = compute(*x_a)
                    load(*x_a)
                    fin = compute(*x_b, carried_finish=fin)
                    load(*x_b)
                    fin = compute(*x_a, carried_finish=fin)
                    load(*x_a)
                    fin = compute(*x_b, carried_finish=fin)
                    fin()

    nc.compile()
    return nc


class _Runner:
    """Compiled 8-core PJRT executable, reusable across calls (no donation)."""

    def __init__(self, nc):
        import jax
        from jax.sharding import Mesh, PartitionSpec
        from jax.experimental.shard_map import shard_map
        from concourse import bass2jax

        bass2jax.install_neuronx_cc_hook()
        self.jax = jax
        self.nc = nc
        partition_name = nc.partition_id_tensor.name if nc.partition_id_tensor else None
        in_names, out_names, out_avals, zero_outs = [], [], [], []
        for alloc in nc.m.functions[0].allocations:
            if not isinstance(alloc, mybir.MemoryLocationSet):
                continue
            name = alloc.memorylocations[0].name
            if alloc.kind == "ExternalInput":
                if name != partition_name:
                    in_names.append(name)
            elif alloc.kind == "ExternalOutput":
                out_names.append(name)
                shape = tuple(alloc.tensor_shape)
                dtype = mybir.dt.np(alloc.dtype)
                out_avals.append(jax.core.ShapedArray(shape, dtype))
                zero_outs.append(np.zeros(shape, dtype))
        self.in_names, self.out_names, self.out_avals = in_names, out_names, out_avals
        all_names = list(in_names) + out_names
        if partition_name is not None:
            all_names.append(partition_name)

        def _body(*args):
            operands = list(args)
            if partition_name is not None:
                operands.append(bass2jax.partition_id_tensor())
            outs = bass2jax._bass_exec_p.bind(
                *operands,
                out_avals=tuple(out_avals),
                in_names=tuple(all_names),
                out_names=tuple(out_names),
                lowering_input_output_aliases=(),
                sim_require_finite=True,
                sim_require_nnan=True,
                nc=nc,
            )
            return tuple(outs)

        devices = jax.devices()[:NCORES]
        mesh = Mesh(np.asarray(devices), ("core",))
        n_params, n_outs = len(in_names), len(out_names)
        in_specs = (PartitionSpec("core"),) * (n_params + n_outs)
        out_specs = (PartitionSpec("core"),) * n_outs
        self.fn = jax.jit(
            shard_map(_body, mesh=mesh, in_specs=in_specs, out_specs=out_specs,
                      check_rep=False),
            keep_unused=True,
        )
        self.zero_args = [
            jax.device_put(np.zeros((NCORES * z.shape[0], *z.shape[1:]), z.dtype))
            for z in zero_outs
        ]

    def prep(self, in_maps):
        per_core = [[np.asarray(m[name]) for name in self.in_names] for m in in_maps]
        concat = [np.concatenate([per_core[c][i] for c in range(NCORES)], axis=0)
                  for i in range(len(self.in_names))]
        return [self.jax.device_put(a) for a in concat] + self.zero_args

    def run(self, args):
        outs = self.fn(*args)
        self.jax.block_until_ready(outs)
        return outs

    def unshard(self, outs):
        return [
            {name: np.asarray(outs[i]).reshape(NCORES, *self.out_avals[i].shape)[c]
             for i, name in enumerate(self.out_names)}
            for c in range(NCORES)
        ]

    def __call__(self, in_maps):
        return self.unshard(self.run(self.prep(in_maps)))


def _get_runner(has_bv: bool, loop_n: int = 1):
    key = (has_bv, loop_n)
    if key not in _CACHE:
        nc = _build(has_bv, loop_n)
        if axon_active():
            _CACHE[key] = _Runner(nc)
        else:
            from concourse.bass_utils import run_bass_kernel_spmd

            def native(in_maps, _nc=nc):
                res = run_bass_kernel_spmd(_nc, in_maps, core_ids=list(range(NCORES)))
                return res.results
            _CACHE[key] = native
    return _CACHE[key]


def _bf16(a):
    import ml_dtypes
    return np.ascontiguousarray(np.asarray(a, dtype=np.float32).astype(ml_dtypes.bfloat16))


def _make_in_maps(inputs):
    wq = np.asarray(inputs["wq"], dtype=np.float32)
    wk = np.asarray(inputs["wk"], dtype=np.float32)
    wv = np.asarray(inputs["wv"], dtype=np.float32)
    bq = np.zeros((128, 1), np.float32)
    bq[:D, 0] = np.asarray(inputs["bq"], dtype=np.float32).ravel()
    bk = np.zeros((128, 1), np.float32)
    bk[:D, 0] = np.asarray(inputs["bk"], dtype=np.float32).ravel()
    bv = np.asarray(inputs["bv"], dtype=np.float32).reshape(1, C)
    wqT = np.zeros((C, 128), np.float32)
    wqT[:, :D] = wq.T
    wkT = np.zeros((C, 128), np.float32)
    wkT[:, :D] = wk.T
    wvT = np.ascontiguousarray(wv.T)
    f1 = _bf16(np.asarray(inputs["feat1"], dtype=np.float32).reshape(B, C, HW))
    f2 = _bf16(np.asarray(inputs["feat2"], dtype=np.float32).reshape(B, C, HW))
    wqTb, wkTb, wvTb, bvb = _bf16(wqT), _bf16(wkT), _bf16(wvT), _bf16(bv)
    in_maps = []
    for core in range(NCORES):
        b, half = divmod(core, 2)
        in_maps.append({
            "x1": np.ascontiguousarray(f1[b][:, IHALF * half:IHALF * (half + 1)]),
            "x2": f2[b],
            "wqT": wqTb, "wkT": wkTb, "wvT": wvTb,
            "bq": bq, "bk": bk, "bv": bvb,
        })
    return in_maps, bool(np.any(bv))


def kernel(**inputs) -> np.ndarray:
    in_maps, has_bv = _make_in_maps(inputs)
    runner = _get_runner(has_bv)
    results = runner(in_maps)
    out = np.empty((B, C, HW), dtype=np.float32)
    for core in range(NCORES):
        b, half = divmod(core, 2)
        out[b][:, IHALF * half:IHALF * (half + 1)] = results[core]["out"]
    return out.reshape(B, C, H, W)
